# revision 32
# baseline (speedup 1.0000x reference)
"""AgreementRouting (CapsNet dynamic routing) Trainium2 kernel.

Data-parallel over batch B=128 across 8 cores (B_local=16 per core).

Per core, u lives in SBUF twice, as fp16:
  u16: partition p = b_loc*16 + d   (b_loc in [0,8), d in [0,16))
       free       = (j in [0,10), h in [0,2), i in [0,1152))
  uT:  partition  = i_lo = i % 128
       free       = (h, j, ci = i//128 in [0,9), p = (b_loc, d))
local batch index beta = h*8 + b_loc.

Structure per routing iteration:
  W1: PE accumulating matmuls a_c = sum_j sfat[j].T @ u16[j]    (fp16)
      (sfat = block-diagonal stationary holding the unnormalized s)
  bb += f80 * a_c                   (DVE STT, squash scale f
                                     folded into the logit update)
  softmax in i-major layout: PE-transpose bb 128-col chunks into
      bbT [i_lo, (ci, j, b)] f32 PSUM, exp on ACT, Z = sum over j via
      strided DVE reduce, c16 = e * recip(Z) with a stride-0 broadcast
      -> cT16 [i_lo, (ci, j, b)] fp16 directly in the transposed layout
  W4: weighted sum on PE with uT as the *stationary*:
      s_ps[(b,d), b'] += sum_i uT[i,(b,d)] * cT16[i,(j,b')]  (9 chunks)
      then a masked DVE reduce extracts the b'==b diagonal into s.
  squash scale f80 computed entirely in (j,b)-partition layout
      (PE matmul + masked STT), no SBUF-shuffle DMAs.

Numerics vs the fp32 oracle: absmax/scale ~ 5e-4.
"""

import os
import sys

import numpy as np

for _p in ("/opt/trn_rl_repo", "/opt/trn_rl_repo/concourse"):
    if _p not in sys.path and os.path.isdir(_p):
        sys.path.insert(0, _p)

B, IC, OC, D = 128, 1152, 10, 16
NCORES = 8
BL = B // NCORES          # 16 local batch
H = 2                     # halves of local batch
BLOC = BL // H            # 8
NI = IC                   # 1152
NC9 = NI // 128           # 9 i-chunks of 128
EPS = 1e-8
NITER = 3
CHUNKS = [(0, 512), (512, 1024), (1024, 1152)]
TIME_REPS = int(os.environ.get("K_TIME_REPS", "1"))  # whole-program reps

_PROG_CACHE = {}


def _build_consts():
    """Host-side constant selector/mask matrices."""
    # base8[(b,d), b2] = 1 if b==b2                      -> [128, 8] f32
    base8 = np.zeros((BLOC * D, BLOC), np.float32)
    for b in range(BLOC):
        base8[b * D:(b + 1) * D, b] = 1.0
    # b82a[(b,d), (j,b2)] = 1 if b==b2                   -> [128, 80]
    b82a = np.tile(base8, (1, OC)).astype(np.float32)
    # b82a3: same mask tiled over (j, piece, b2)         -> [128, 240]
    b82a3 = np.tile(base8, (1, OC * 3)).astype(np.float32)
    # jmask[(j,b), j2] = 1 if j==j2                      -> [80, 10] f32
    jmask = np.zeros((OC * BLOC, OC), np.float32)
    for j in range(OC):
        jmask[j * BLOC:(j + 1) * BLOC, j] = 1.0
    ident80 = np.eye(OC * BLOC, dtype=np.float32)
    return dict(base8=base8,
                b82a3_32=b82a3, b82a16=b82a.astype(np.float16),
                b82aT16=np.ascontiguousarray(b82a.T).astype(np.float16),
                jmask=jmask, ident80=ident80)


def _build_program(general_b):
    import concourse.bacc as bacc
    import concourse.mybir as mybir
    import concourse.tile as tile

    dt = mybir.dt
    AF = mybir.ActivationFunctionType
    ALU = mybir.AluOpType
    AX = mybir.AxisListType

    # Force a single shared ACT table (Exp+Ln+Copy+Identity in one set) so
    # the table-load pass emits one load instead of thrashing per func.
    from concourse import hw_specs as _hws
    _orig_tabs = _hws.get_activation_tables
    _keep = "natural_log_exp_and_others"

    def _patched_tabs(arch, __orig=_orig_tabs, __keep=_keep):
        tabs = __orig(arch)
        return {n: (s if n == __keep else set()) for n, s in tabs.items()}

    bacc.get_activation_tables = _patched_tabs

    nc = bacc.Bacc("TRN2", target_bir_lowering=False, debug=False)

    # ---- DRAM I/O ----
    u16_d = nc.dram_tensor("u16", [128, OC, H, NI], dt.float16,
                           kind="ExternalInput").ap()
    uT_d = nc.dram_tensor("uT", [128, H, OC, NC9 * 128], dt.float16,
                          kind="ExternalInput").ap()
    base8_d = nc.dram_tensor("base8", [BLOC * D, BLOC], dt.float32,
                             kind="ExternalInput").ap()
    b82a3_d = nc.dram_tensor("b82a3_32", [BLOC * D, 3 * OC * BLOC], dt.float32,
                             kind="ExternalInput").ap()
    b82a16_d = nc.dram_tensor("b82a16", [BLOC * D, OC * BLOC], dt.float16,
                              kind="ExternalInput").ap()
    b82aT16_d = nc.dram_tensor("b82aT16", [OC * BLOC, BLOC * D], dt.float16,
                               kind="ExternalInput").ap()
    jmask_d = nc.dram_tensor("jmask", [OC * BLOC, OC], dt.float32,
                             kind="ExternalInput").ap()
    ident80_d = nc.dram_tensor("ident80", [OC * BLOC, OC * BLOC], dt.float32,
                               kind="ExternalInput").ap()
    if general_b:
        c0_d = nc.dram_tensor("c0rep", [128, OC, NI], dt.float16,
                              kind="ExternalInput").ap()
        bb0_d = nc.dram_tensor("bb0", [OC * BLOC, NI], dt.float32,
                               kind="ExternalInput").ap()
    out_d = nc.dram_tensor("vout", [128, 2 * OC], dt.float32,
                           kind="ExternalOutput").ap()

    # ---- static SBUF ----
    def sb(name, shape, dtype):
        return nc.alloc_sbuf_tensor(name, list(shape), dtype).ap()

    u16 = sb("u16_sb", [128, OC * H * NI], dt.float16)       # 46KB/part
    uT = sb("uT_sb", [128, H * OC * NC9 * 128], dt.float16)  # 46KB/part
    base8_sb = sb("base8_sb", [BLOC * D, BLOC], dt.float32)
    b82a3_sb = sb("b82a3_sb", [BLOC * D, 3 * OC * BLOC], dt.float32)
    b82a16_sb = sb("b82a16_sb", [BLOC * D, OC * BLOC], dt.float16)
    b82aT16_sb = sb("b82aT16_sb", [OC * BLOC, BLOC * D], dt.float16)
    jmask_sb = sb("jmask_sb", [OC * BLOC, OC], dt.float32)
    ident80_sb = sb("ident80_sb", [OC * BLOC, OC * BLOC], dt.float32)
    bb = [sb(f"bbsb{h}", [OC * BLOC, NI], dt.float32) for h in range(H)]
    sfat = [[sb(f"sfat{j}_{h}", [128, OC * BLOC], dt.float16)
             for h in range(H)] for j in range(OC)]
    f80 = [sb(f"f80_{h}", [OC * BLOC, 1], dt.float32) for h in range(H)]
    s_sb = sb("s_sb", [128, H * OC], dt.float32)
    # mini-squash scratch (per h), all in [80, *] layout
    ssqh = [sb(f"ssqh{h}", [128, OC], dt.float16) for h in range(H)]
    jscr = [sb(f"jscr{h}", [OC * BLOC, OC], dt.float32) for h in range(H)]
    sqe80 = [sb(f"sqe80_{h}", [OC * BLOC, 1], dt.float32) for h in range(H)]
    lnx80 = [sb(f"lnx80_{h}", [OC * BLOC, 1], dt.float32) for h in range(H)]
    r80 = [sb(f"r80_{h}", [OC * BLOC, 1], dt.float32) for h in range(H)]
    den80 = [sb(f"den80_{h}", [OC * BLOC, 1], dt.float32) for h in range(H)]
    rec80 = [sb(f"rec80_{h}", [OC * BLOC, 1], dt.float32) for h in range(H)]
    # W4 extraction scratch
    mskd = [sb(f"mskd{h}", [128, 3 * OC * BLOC], dt.float32) for h in range(H)]
    # final squash scratch
    fj16 = [sb(f"fj16_{h}", [OC * BLOC, OC], dt.float16) for h in range(H)]
    v_sb = sb("v_sb", [128, H * OC], dt.float32)

    def uview(j, h):
        off = (j * H + h) * NI
        return u16[:, off:off + NI]

    def uTview(h, j, ci):
        off = ((h * OC + j) * NC9 + ci) * 128
        return uT[:, off:off + 128]

    with tile.TileContext(nc) as tc:
        from contextlib import ExitStack
        with ExitStack() as ctx:
            psA = ctx.enter_context(
                tc.tile_pool(name="psA", bufs=3, space="PSUM"))
            psB = ctx.enter_context(
                tc.tile_pool(name="psB", bufs=2, space="PSUM"))
            psS = ctx.enter_context(
                tc.tile_pool(name="psS", bufs=2, space="PSUM"))
            sc = ctx.enter_context(
                tc.tile_pool(name="sc", bufs=int(os.environ.get("K_SCBUFS", "3"))))
            ec = ctx.enter_context(
                tc.tile_pool(name="ec", bufs=int(os.environ.get("K_ECBUFS", "3"))))

            for _rep in range(TIME_REPS):
                # ---- loads: u16 first (gates init+W1), consts, then uT ----
                for j in range(OC):
                    for h in range(H):
                        off = (j * H + h) * NI
                        nc.sync.dma_start(
                            u16[:, off:off + NI], u16_d[:, j, h, :])
                nc.sync.dma_start(base8_sb[:], base8_d)
                nc.sync.dma_start(b82a3_sb[:], b82a3_d)
                nc.sync.dma_start(b82a16_sb[:], b82a16_d)
                nc.sync.dma_start(b82aT16_sb[:], b82aT16_d)
                nc.sync.dma_start(jmask_sb[:], jmask_d)
                nc.sync.dma_start(ident80_sb[:], ident80_d)
                for h in range(H):
                    for j in range(OC):
                        off = (h * OC + j) * NC9 * 128
                        nc.sync.dma_start(
                            uT[:, off:off + NC9 * 128], uT_d[:, h, j, :])

                # ---- init bb and sfat ----
                for h in range(H):
                    if general_b:
                        nc.sync.dma_start(bb[h][:], bb0_d)
                    else:
                        nc.gpsimd.memset(bb[h][:], 0.0)
                for j in range(OC):
                    for h in range(H):
                        nc.gpsimd.memset(sfat[j][h][:], 0.0)

                def build_sfat(j, h):
                    # ACT: Copy(base8 * s_col) with a per-partition scale
                    col = 2 * j + h
                    nc.scalar.mul(
                        sfat[j][h][:, j * BLOC:(j + 1) * BLOC],
                        base8_sb[:], s_sb[:, col:col + 1])

                def mini_squash(h):
                    """f80[h] <- squash scale, computed in [80,*] layout.

                    f = sq/((1+sq)*sqrt(sq+EPS)); sqe = sq+EPS stands in for
                    sq (EPS=1e-8 absolute, negligible).
                    """
                    s_h = s_sb[:, h::2]  # [128, OC] strided view
                    nc.scalar.activation(ssqh[h][:], s_h, AF.Square)
                    sq_ps = psB.tile([OC * BLOC, OC], dt.float32, tag="bank",
                                     name="sq80_ps")
                    nc.tensor.matmul(sq_ps[:], b82a16_sb[:], ssqh[h][:],
                                     start=True, stop=True)
                    # sqe = EPS + sum_j'(sq_ps * jmask)  (one fused op)
                    nc.vector.tensor_tensor_reduce(
                        out=jscr[h][:], in0=sq_ps[:], in1=jmask_sb[:],
                        scale=1.0, scalar=EPS, op0=ALU.mult, op1=ALU.add,
                        accum_out=sqe80[h][:])
                    nc.scalar.activation(lnx80[h][:], sqe80[h][:], AF.Ln)
                    nc.scalar.activation(r80[h][:], lnx80[h][:], AF.Exp,
                                         scale=0.5)
                    # den = (sqe + 1) * r
                    nc.vector.tensor_scalar(
                        out=den80[h][:], in0=sqe80[h][:],
                        scalar1=1.0, scalar2=r80[h][:, 0:1],
                        op0=ALU.add, op1=ALU.mult)
                    nc.vector.reciprocal(rec80[h][:], den80[h][:])
                    nc.vector.tensor_scalar(
                        out=f80[h][:], in0=sqe80[h][:],
                        scalar1=rec80[h][:, 0:1], scalar2=None, op0=ALU.mult)

                # ---- init s0 ----
                if general_b:
                    c0_sb = sc.tile([128, OC * NI], dt.float16, tag="c0",
                                    name="c0_sb", bufs=1)
                    nc.sync.dma_start(c0_sb[:], c0_d)
                    for j in range(OC):
                        for h in range(H):
                            col = 2 * j + h
                            scr = sc.tile([128, NI], dt.float16, tag="scr",
                                          name="scr")
                            nc.vector.scalar_tensor_tensor(
                                out=scr[:], in0=uview(j, h), scalar=1.0,
                                in1=c0_sb[:, j * NI:(j + 1) * NI],
                                op0=ALU.mult, op1=ALU.mult,
                                accum_out=s_sb[:, col:col + 1])
                else:
                    # split the 20 (j,h) reduction units across DVE/ACT/GpSimd
                    units = [(j, h) for j in range(OC) for h in range(H)]
                    for idx, (j, h) in enumerate(units):
                        col = 2 * j + h
                        eng = ("dve", "act")[idx % 2]
                        if eng == "dve":
                            nc.vector.reduce_sum(
                                s_sb[:, col:col + 1], uview(j, h), axis=AX.X)
                        else:
                            scr = sc.tile([128, NI], dt.float16, tag="scr",
                                          name="scr")
                            nc.scalar.activation(
                                scr[:], uview(j, h), AF.Identity,
                                accum_out=s_sb[:, col:col + 1])
                if not general_b:
                    nc.vector.tensor_scalar_mul(s_sb[:], s_sb[:], 1.0 / OC)
                for j in range(OC):
                    for h in range(H):
                        build_sfat(j, h)
                for h in range(H):
                    mini_squash(h)

                # ---- routing iterations ----
                # Pipeline pieces: unit (h, c) covers i-cols CHUNKS[c] =
                # transpose-chunks TCH[c]. Stage-major emission per stage so
                # cross-engine latency amortizes; W1 and the bb-transposes
                # interleave on PE so the softmax pipeline fills while W1
                # still streams.
                TCH = [(0, 4), (4, 8), (8, 9)]
                hc = [(h, ci) for h in range(H) for ci in range(len(CHUNKS))]

                for it in range(NITER):
                    last = it == NITER - 1
                    a_t, bbT_t, eT, zrT, cT16 = {}, {}, {}, {}, {}
                    for h in range(H):
                        eT[h] = ec.tile([128, NC9 * 80], dt.float32,
                                        tag="eT", name="eT")
                        zrT[h] = ec.tile([128, NC9 * BLOC], dt.float32,
                                         tag="zrT", name="zrT")
                        cT16[h] = ec.tile([128, NC9 * 80], dt.float16,
                                          tag="ct", name="cT16")

                    def emit_w1(h, ci):
                        c0, c1 = CHUNKS[ci]
                        a_c = psB.tile([OC * BLOC, 512], dt.float32,
                                       tag="bank", name="a_c")
                        for j in range(OC):
                            nc.tensor.matmul(
                                a_c[:, :c1 - c0], sfat[j][h],
                                uview(j, h)[:, c0:c1],
                                start=(j == 0), stop=(j == OC - 1))
                        a_t[(h, ci)] = a_c
                        # GPSIMD cannot access PSUM (a_c), so DVE only here
                        nc.vector.scalar_tensor_tensor(
                            out=bb[h][:, c0:c1], in0=a_c[:, :c1 - c0],
                            scalar=f80[h][:, 0:1], in1=bb[h][:, c0:c1],
                            op0=ALU.mult, op1=ALU.add)

                    def emit_transp(h, c):
                        t0, t1 = TCH[c]
                        bbT = psA.tile([128, 320], dt.float32,
                                       tag="bbT", name="bbT")
                        for ci in range(t0, t1):
                            nc.tensor.transpose(
                                bbT[:, (ci - t0) * 80:(ci - t0 + 1) * 80],
                                bb[h][:, ci * 128:(ci + 1) * 128],
                                ident80_sb[:])
                        bbT_t[(h, c)] = bbT

                    for (h, ci) in hc:
                        emit_w1(h, ci)
                    for (h, c) in hc:
                        emit_transp(h, c)

                    for (h, c) in hc:
                        t0, t1 = TCH[c]
                        nc.scalar.activation(
                            eT[h][:, t0 * 80:t1 * 80],
                            bbT_t[(h, c)][:, :(t1 - t0) * 80], AF.Exp)
                    for (h, c) in hc:
                        t0, t1 = TCH[c]
                        ncc = t1 - t0
                        nc.vector.reduce_sum(
                            zrT[h][:, t0 * BLOC:t1 * BLOC].rearrange(
                                "p (ci b) -> p ci b", ci=ncc),
                            eT[h][:, t0 * 80:t1 * 80].rearrange(
                                "p (ci j b) -> p ci b j", ci=ncc, j=OC),
                            axis=AX.X)
                        nc.vector.reciprocal(
                            zrT[h][:, t0 * BLOC:t1 * BLOC],
                            zrT[h][:, t0 * BLOC:t1 * BLOC])
                    for (h, c) in hc:
                        t0, t1 = TCH[c]
                        ncc = t1 - t0
                        eng = nc.gpsimd if h == 1 else nc.vector
                        eng.tensor_tensor(
                            cT16[h][:, t0 * 80:t1 * 80].rearrange(
                                "p (ci j b) -> p ci j b", ci=ncc, j=OC),
                            eT[h][:, t0 * 80:t1 * 80].rearrange(
                                "p (ci j b) -> p ci j b", ci=ncc, j=OC),
                            zrT[h][:, t0 * BLOC:t1 * BLOC].rearrange(
                                "p (ci b) -> p ci b", ci=ncc)[:, :, None, :]
                            .broadcast_to([128, ncc, OC, BLOC]),
                            op=ALU.mult)

                    # ---- W4: PE weighted-sum with uT stationary; masked
                    # reduce extracts the b'==b diagonal ----
                    # each (j, piece) group opens and closes within its piece
                    # (CoreSim allows only one open group per psum bank);
                    # piece partials land at columns (j, c, b') of s_ps.
                    s_ps_t = {}
                    for h in range(H):
                        s_ps_t[h] = psS.tile([128, 3 * OC * BLOC], dt.float32,
                                             tag="sps", name="s_ps")
                    for (h, c) in hc:
                        t0, t1 = TCH[c]
                        for j in range(OC):
                            col = (j * 3 + c) * BLOC
                            for ci in range(t0, t1):
                                nc.tensor.matmul(
                                    s_ps_t[h][:, col:col + BLOC],
                                    uTview(h, j, ci),
                                    cT16[h][:, ci * 80 + j * BLOC:
                                            ci * 80 + (j + 1) * BLOC],
                                    start=(ci == t0), stop=(ci == t1 - 1))
                    for h in range(H):
                        nc.vector.tensor_tensor(mskd[h][:], s_ps_t[h][:],
                                                b82a3_sb[:], op=ALU.mult)
                        nc.vector.reduce_sum(
                            s_sb[:, h::2],
                            mskd[h][:].rearrange("p (j cb) -> p j cb", j=OC),
                            axis=AX.X)
                        if not last:
                            for j in range(OC):
                                build_sfat(j, h)
                        mini_squash(h)
                        if last:
                            # final: v = f * s, with f replicated from [80,1]
                            # to [(b,d), j] via jmask scale + b82aT matmul
                            nc.vector.tensor_scalar(
                                out=fj16[h][:], in0=jmask_sb[:],
                                scalar1=f80[h][:, 0:1], scalar2=None,
                                op0=ALU.mult)
                            f_ps = psB.tile([128, OC], dt.float32,
                                            tag="bank", name="f_ps")
                            nc.tensor.matmul(f_ps[:], b82aT16_sb[:],
                                             fj16[h][:], start=True, stop=True)
                            nc.vector.tensor_tensor(
                                v_sb[:, h::2], s_sb[:, h::2], f_ps[:],
                                op=ALU.mult)
                nc.sync.dma_start(out_d, v_sb[:])

    nc.compile()
    return nc


def _get_program(general_b):
    key = bool(general_b)
    if key not in _PROG_CACHE:
        _PROG_CACHE[key] = _build_program(key)
    return _PROG_CACHE[key]


def _prep_inputs(u_predict, b):
    """Host-side shard + layout transform. Returns (in_maps, general_b)."""
    general_b = bool(np.any(b != 0.0))
    consts = _build_consts()
    u16 = u_predict.astype(np.float16)
    u6 = u16.reshape(NCORES, H, BLOC, IC, OC, D)
    ut = np.ascontiguousarray(u6.transpose(0, 2, 5, 4, 1, 3))
    ut = ut.reshape(NCORES, 128, OC, H, NI)
    # uT[c, i_lo, h, j, ci*128 + p] = ut[c, p, j, h, ci*128 + i_lo]
    u5 = ut.reshape(NCORES, 128, OC, H, NC9, 128)
    uTt = np.ascontiguousarray(u5.transpose(0, 5, 3, 2, 4, 1))
    uTt = uTt.reshape(NCORES, 128, H, OC, NC9 * 128)

    extra = {}
    if general_b:
        bm = b.astype(np.float64)
        e = np.exp(bm - bm.max(axis=1, keepdims=True))
        c0 = (e / e.sum(axis=1, keepdims=True)).astype(np.float16)  # [IC, OC]
        c0rep = np.ascontiguousarray(
            np.broadcast_to(c0.T[None, :, :], (128, OC, NI))).astype(
                np.float16)
        bt = b.astype(np.float32).T  # [OC, NI]
        bb0 = np.ascontiguousarray(
            np.repeat(bt[:, None, :], BLOC, axis=1)).reshape(OC * BLOC, NI)
        extra = {"c0rep": c0rep, "bb0": bb0}

    in_maps = []
    for c in range(NCORES):
        m = {"u16": ut[c], "uT": uTt[c]}
        m.update(consts)
        m.update(extra)
        in_maps.append(m)
    return in_maps, general_b


def _gather_output(results):
    out = np.empty((B, OC, D), np.float32)
    for c in range(NCORES):
        v = results[c]["vout"]                  # [p=(bl,d), col=(j*2+h)]
        v4 = v.reshape(BLOC, D, OC, H)          # bl, d, j, h
        out[c * BL:(c + 1) * BL] = v4.transpose(3, 0, 2, 1).reshape(
            BL, OC, D)
    return out


def kernel(u_predict, b=None, **kw):
    u_predict = np.asarray(u_predict, dtype=np.float32)
    if b is None:
        b = np.zeros((IC, OC), np.float32)
    b = np.asarray(b, dtype=np.float32)
    in_maps, general_b = _prep_inputs(u_predict, b)
    nc = _get_program(general_b)

    if os.environ.get("BASS_KERNEL_SIM"):
        from concourse.bass_interp import CoreSim
        sim = CoreSim(nc, trace=False)
        for name, arr in in_maps[0].items():
            sim.tensor(name)[:] = arr
        sim.simulate(check_with_hw=False)
        v0 = np.array(sim.tensor("vout"))
        out = np.empty((B, OC, D), np.float32)
        v4 = v0.reshape(BLOC, D, OC, H)
        out[:BL] = v4.transpose(3, 0, 2, 1).reshape(BL, OC, D)
        return out  # NOTE: only core 0 valid in sim mode

    from concourse import bass_utils
    trace = bool(os.environ.get("BASS_KERNEL_TRACE"))
    res = bass_utils.run_bass_kernel_spmd(
        nc, in_maps, core_ids=list(range(NCORES)), trace=trace)
    kernel.last_results = res
    return _gather_output(res.results)


# revision 35
# speedup vs baseline: 1.2155x; 1.2155x over previous
"""AgreementRouting (CapsNet dynamic routing) Trainium2 kernel.

Data-parallel over batch B=128 across 8 cores (B_local=16 per core).

Per core, u lives in SBUF twice, as fp16:
  u16: partition p = b_loc*16 + d   (b_loc in [0,8), d in [0,16))
       free       = (j in [0,10), h in [0,2), i in [0,1152))
  uT:  partition  = i_lo = i % 128
       free       = (h, j, ci = i//128 in [0,9), p = (b_loc, d))
local batch index beta = h*8 + b_loc.

Structure per routing iteration:
  W1: PE accumulating matmuls a_c = sum_j sfat[j].T @ u16[j]    (fp16)
      (sfat = block-diagonal stationary holding the unnormalized s)
  bb += f80 * a_c                   (DVE STT, squash scale f
                                     folded into the logit update)
  softmax in i-major layout: PE-transpose bb 128-col chunks into
      bbT [i_lo, (ci, j, b)] f32 PSUM, exp on ACT, Z = sum over j via
      strided DVE reduce, c16 = e * recip(Z) with a stride-0 broadcast
      -> cT16 [i_lo, (ci, j, b)] fp16 directly in the transposed layout
  W4: weighted sum on PE with uT as the *stationary*:
      s_ps[(b,d), b'] += sum_i uT[i,(b,d)] * cT16[i,(j,b')]  (9 chunks)
      then a masked DVE reduce extracts the b'==b diagonal into s.
  squash scale f80 computed entirely in (j,b)-partition layout
      (PE matmul + masked STT), no SBUF-shuffle DMAs.

Numerics vs the fp32 oracle: absmax/scale ~ 5e-4.
"""

import os
import sys

import numpy as np

for _p in ("/opt/trn_rl_repo", "/opt/trn_rl_repo/concourse"):
    if _p not in sys.path and os.path.isdir(_p):
        sys.path.insert(0, _p)

B, IC, OC, D = 128, 1152, 10, 16
NCORES = 8
BL = B // NCORES          # 16 local batch
H = 2                     # halves of local batch
BLOC = BL // H            # 8
NI = IC                   # 1152
NC9 = NI // 128           # 9 i-chunks of 128
EPS = 1e-8
NITER = 3
CHUNKS = [(0, 512), (512, 1024), (1024, 1152)]
TIME_REPS = int(os.environ.get("K_TIME_REPS", "1"))  # whole-program reps

_PROG_CACHE = {}


def _build_consts():
    """Host-side constant selector/mask matrices."""
    # base8[(b,d), b2] = 1 if b==b2                      -> [128, 8] f32
    base8 = np.zeros((BLOC * D, BLOC), np.float32)
    for b in range(BLOC):
        base8[b * D:(b + 1) * D, b] = 1.0
    # b82a[(b,d), (j,b2)] = 1 if b==b2                   -> [128, 80]
    b82a = np.tile(base8, (1, OC)).astype(np.float32)
    # b82a3: same mask tiled over (j, piece, b2)         -> [128, 240]
    b82a3 = np.tile(base8, (1, OC * 3)).astype(np.float32)
    # jmask[(j,b), j2] = 1 if j==j2                      -> [80, 10] f32
    jmask = np.zeros((OC * BLOC, OC), np.float32)
    for j in range(OC):
        jmask[j * BLOC:(j + 1) * BLOC, j] = 1.0
    ident80 = np.eye(OC * BLOC, dtype=np.float32)
    return dict(base8=base8,
                b82a3_32=b82a3, b82a16=b82a.astype(np.float16),
                b82aT16=np.ascontiguousarray(b82a.T).astype(np.float16),
                jmask=jmask, ident80=ident80)


def _build_program(general_b):
    import concourse.bacc as bacc
    import concourse.mybir as mybir
    import concourse.tile as tile

    dt = mybir.dt
    AF = mybir.ActivationFunctionType
    ALU = mybir.AluOpType
    AX = mybir.AxisListType

    # Force a single shared ACT table (Exp+Ln+Copy+Identity in one set) so
    # the table-load pass emits one load instead of thrashing per func.
    from concourse import hw_specs as _hws
    _orig_tabs = _hws.get_activation_tables
    _keep = "natural_log_exp_and_others"

    def _patched_tabs(arch, __orig=_orig_tabs, __keep=_keep):
        tabs = __orig(arch)
        return {n: (s if n == __keep else set()) for n, s in tabs.items()}

    bacc.get_activation_tables = _patched_tabs

    nc = bacc.Bacc("TRN2", target_bir_lowering=False, debug=False)

    # ---- DRAM I/O ----
    u16_d = nc.dram_tensor("u16", [128, OC, H, NI], dt.float16,
                           kind="ExternalInput").ap()
    uT_d = nc.dram_tensor("uT", [128, H, OC, NC9 * 128], dt.float16,
                          kind="ExternalInput").ap()
    base8_d = nc.dram_tensor("base8", [BLOC * D, BLOC], dt.float32,
                             kind="ExternalInput").ap()
    b82a3_d = nc.dram_tensor("b82a3_32", [BLOC * D, 3 * OC * BLOC], dt.float32,
                             kind="ExternalInput").ap()
    b82a16_d = nc.dram_tensor("b82a16", [BLOC * D, OC * BLOC], dt.float16,
                              kind="ExternalInput").ap()
    b82aT16_d = nc.dram_tensor("b82aT16", [OC * BLOC, BLOC * D], dt.float16,
                               kind="ExternalInput").ap()
    jmask_d = nc.dram_tensor("jmask", [OC * BLOC, OC], dt.float32,
                             kind="ExternalInput").ap()
    ident80_d = nc.dram_tensor("ident80", [OC * BLOC, OC * BLOC], dt.float32,
                               kind="ExternalInput").ap()
    if general_b:
        c0_d = nc.dram_tensor("c0rep", [128, OC, NI], dt.float16,
                              kind="ExternalInput").ap()
        bb0_d = nc.dram_tensor("bb0", [OC * BLOC, NI], dt.float32,
                               kind="ExternalInput").ap()
    out_d = nc.dram_tensor("vout", [128, 2 * OC], dt.float32,
                           kind="ExternalOutput").ap()

    # ---- static SBUF ----
    def sb(name, shape, dtype):
        return nc.alloc_sbuf_tensor(name, list(shape), dtype).ap()

    u16 = sb("u16_sb", [128, OC * H * NI], dt.float16)       # 46KB/part
    uT = sb("uT_sb", [128, H * OC * NC9 * 128], dt.float16)  # 46KB/part
    base8_sb = sb("base8_sb", [BLOC * D, BLOC], dt.float32)
    b82a3_sb = sb("b82a3_sb", [BLOC * D, 3 * OC * BLOC], dt.float32)
    b82a16_sb = sb("b82a16_sb", [BLOC * D, OC * BLOC], dt.float16)
    b82aT16_sb = sb("b82aT16_sb", [OC * BLOC, BLOC * D], dt.float16)
    jmask_sb = sb("jmask_sb", [OC * BLOC, OC], dt.float32)
    ident80_sb = sb("ident80_sb", [OC * BLOC, OC * BLOC], dt.float32)
    bb = [sb(f"bbsb{h}", [OC * BLOC, NI], dt.float32) for h in range(H)]
    sfat = [[sb(f"sfat{j}_{h}", [128, OC * BLOC], dt.float16)
             for h in range(H)] for j in range(OC)]
    f80 = [sb(f"f80_{h}", [OC * BLOC, 1], dt.float32) for h in range(H)]
    s_sb = sb("s_sb", [128, H * OC], dt.float32)
    # mini-squash scratch (per h), all in [80, *] layout
    ssqh = [sb(f"ssqh{h}", [128, OC], dt.float16) for h in range(H)]
    jscr = [sb(f"jscr{h}", [OC * BLOC, OC], dt.float32) for h in range(H)]
    sqe80 = [sb(f"sqe80_{h}", [OC * BLOC, 1], dt.float32) for h in range(H)]
    lnx80 = [sb(f"lnx80_{h}", [OC * BLOC, 1], dt.float32) for h in range(H)]
    r80 = [sb(f"r80_{h}", [OC * BLOC, 1], dt.float32) for h in range(H)]
    den80 = [sb(f"den80_{h}", [OC * BLOC, 1], dt.float32) for h in range(H)]
    rec80 = [sb(f"rec80_{h}", [OC * BLOC, 1], dt.float32) for h in range(H)]
    # W4 extraction scratch
    mskd = [sb(f"mskd{h}", [128, 3 * OC * BLOC], dt.float32) for h in range(H)]
    # final squash scratch
    fj16 = [sb(f"fj16_{h}", [OC * BLOC, OC], dt.float16) for h in range(H)]
    v_sb = sb("v_sb", [128, H * OC], dt.float32)

    def uview(j, h):
        off = (j * H + h) * NI
        return u16[:, off:off + NI]

    def uTview(h, j, ci):
        off = ((h * OC + j) * NC9 + ci) * 128
        return uT[:, off:off + 128]

    with tile.TileContext(nc) as tc:
        from contextlib import ExitStack
        with ExitStack() as ctx:
            psA = ctx.enter_context(
                tc.tile_pool(name="psA", bufs=3, space="PSUM"))
            psB = ctx.enter_context(
                tc.tile_pool(name="psB", bufs=2, space="PSUM"))
            psS = ctx.enter_context(
                tc.tile_pool(name="psS", bufs=2, space="PSUM"))
            sc = ctx.enter_context(
                tc.tile_pool(name="sc", bufs=int(os.environ.get("K_SCBUFS", "3"))))
            ec = ctx.enter_context(
                tc.tile_pool(name="ec", bufs=int(os.environ.get("K_ECBUFS", "3"))))

            for _rep in range(TIME_REPS):
                # ---- loads: u16 first (gates init+W1), consts, then uT ----
                for j in range(OC):
                    for h in range(H):
                        off = (j * H + h) * NI
                        nc.sync.dma_start(
                            u16[:, off:off + NI], u16_d[:, j, h, :])
                nc.sync.dma_start(base8_sb[:], base8_d)
                nc.sync.dma_start(b82a3_sb[:], b82a3_d)
                nc.sync.dma_start(b82a16_sb[:], b82a16_d)
                nc.sync.dma_start(b82aT16_sb[:], b82aT16_d)
                nc.sync.dma_start(jmask_sb[:], jmask_d)
                nc.sync.dma_start(ident80_sb[:], ident80_d)
                for h in range(H):
                    for j in range(OC):
                        off = (h * OC + j) * NC9 * 128
                        nc.sync.dma_start(
                            uT[:, off:off + NC9 * 128], uT_d[:, h, j, :])

                # ---- init bb and sfat ----
                for h in range(H):
                    if general_b:
                        nc.sync.dma_start(bb[h][:], bb0_d)
                    else:
                        nc.gpsimd.memset(bb[h][:], 0.0)
                for j in range(OC):
                    for h in range(H):
                        nc.gpsimd.memset(sfat[j][h][:], 0.0)

                def build_sfat(j, h):
                    col = 2 * j + h
                    nc.vector.tensor_scalar(
                        out=sfat[j][h][:, j * BLOC:(j + 1) * BLOC],
                        in0=base8_sb[:],
                        scalar1=s_sb[:, col:col + 1],
                        scalar2=None,
                        op0=ALU.mult)

                def mini_squash(h):
                    """f80[h] <- squash scale, computed in [80,*] layout.

                    f = sq/((1+sq)*sqrt(sq+EPS)); sqe = sq+EPS stands in for
                    sq (EPS=1e-8 absolute, negligible).
                    """
                    s_h = s_sb[:, h::2]  # [128, OC] strided view
                    nc.vector.tensor_tensor(ssqh[h][:], s_h, s_h, op=ALU.mult)
                    sq_ps = psB.tile([OC * BLOC, OC], dt.float32, tag="bank",
                                     name="sq80_ps")
                    nc.tensor.matmul(sq_ps[:], b82a16_sb[:], ssqh[h][:],
                                     start=True, stop=True)
                    # sqe = EPS + sum_j'(sq_ps * jmask)  (one fused op)
                    nc.vector.tensor_tensor_reduce(
                        out=jscr[h][:], in0=sq_ps[:], in1=jmask_sb[:],
                        scale=1.0, scalar=EPS, op0=ALU.mult, op1=ALU.add,
                        accum_out=sqe80[h][:])
                    nc.scalar.activation(lnx80[h][:], sqe80[h][:], AF.Ln)
                    nc.scalar.activation(r80[h][:], lnx80[h][:], AF.Exp,
                                         scale=0.5)
                    # den = (sqe + 1) * r
                    nc.vector.tensor_scalar(
                        out=den80[h][:], in0=sqe80[h][:],
                        scalar1=1.0, scalar2=r80[h][:, 0:1],
                        op0=ALU.add, op1=ALU.mult)
                    nc.vector.reciprocal(rec80[h][:], den80[h][:])
                    nc.vector.tensor_scalar(
                        out=f80[h][:], in0=sqe80[h][:],
                        scalar1=rec80[h][:, 0:1], scalar2=None, op0=ALU.mult)

                # ---- init s0 ----
                if general_b:
                    c0_sb = sc.tile([128, OC * NI], dt.float16, tag="c0",
                                    name="c0_sb", bufs=1)
                    nc.sync.dma_start(c0_sb[:], c0_d)
                    for j in range(OC):
                        for h in range(H):
                            col = 2 * j + h
                            scr = sc.tile([128, NI], dt.float16, tag="scr",
                                          name="scr")
                            nc.vector.scalar_tensor_tensor(
                                out=scr[:], in0=uview(j, h), scalar=1.0,
                                in1=c0_sb[:, j * NI:(j + 1) * NI],
                                op0=ALU.mult, op1=ALU.mult,
                                accum_out=s_sb[:, col:col + 1])
                else:
                    # split the 20 (j,h) reduction units across DVE/ACT/GpSimd
                    units = [(j, h) for j in range(OC) for h in range(H)]
                    for idx, (j, h) in enumerate(units):
                        col = 2 * j + h
                        eng = ("dve", "act")[idx % 2]
                        if eng == "dve":
                            nc.vector.reduce_sum(
                                s_sb[:, col:col + 1], uview(j, h), axis=AX.X)
                        else:
                            scr = sc.tile([128, NI], dt.float16, tag="scr",
                                          name="scr")
                            nc.scalar.activation(
                                scr[:], uview(j, h), AF.Identity,
                                accum_out=s_sb[:, col:col + 1])
                if not general_b:
                    nc.vector.tensor_scalar_mul(s_sb[:], s_sb[:], 1.0 / OC)
                for j in range(OC):
                    for h in range(H):
                        build_sfat(j, h)
                for h in range(H):
                    mini_squash(h)

                # ---- routing iterations ----
                # Pipeline pieces: unit (h, c) covers i-cols CHUNKS[c] =
                # transpose-chunks TCH[c]. Stage-major emission per stage so
                # cross-engine latency amortizes; W1 and the bb-transposes
                # interleave on PE so the softmax pipeline fills while W1
                # still streams.
                TCH = [(0, 4), (4, 8), (8, 9)]
                hc = [(h, ci) for h in range(H) for ci in range(len(CHUNKS))]

                for it in range(NITER):
                    last = it == NITER - 1
                    a_t, bbT_t, eT, zrT, cT16 = {}, {}, {}, {}, {}
                    for h in range(H):
                        eT[h] = ec.tile([128, NC9 * 80], dt.float32,
                                        tag="eT", name="eT")
                        zrT[h] = ec.tile([128, NC9 * BLOC], dt.float32,
                                         tag="zrT", name="zrT")
                        cT16[h] = ec.tile([128, NC9 * 80], dt.float16,
                                          tag="ct", name="cT16")

                    def emit_w1(h, ci):
                        c0, c1 = CHUNKS[ci]
                        a_c = psB.tile([OC * BLOC, 512], dt.float32,
                                       tag="bank", name="a_c")
                        for j in range(OC):
                            nc.tensor.matmul(
                                a_c[:, :c1 - c0], sfat[j][h],
                                uview(j, h)[:, c0:c1],
                                start=(j == 0), stop=(j == OC - 1))
                        a_t[(h, ci)] = a_c
                        # GPSIMD cannot access PSUM (a_c), so DVE only here
                        nc.vector.scalar_tensor_tensor(
                            out=bb[h][:, c0:c1], in0=a_c[:, :c1 - c0],
                            scalar=f80[h][:, 0:1], in1=bb[h][:, c0:c1],
                            op0=ALU.mult, op1=ALU.add)

                    def emit_transp(h, c):
                        t0, t1 = TCH[c]
                        bbT = psA.tile([128, 320], dt.float32,
                                       tag="bbT", name="bbT")
                        for ci in range(t0, t1):
                            nc.tensor.transpose(
                                bbT[:, (ci - t0) * 80:(ci - t0 + 1) * 80],
                                bb[h][:, ci * 128:(ci + 1) * 128],
                                ident80_sb[:])
                        bbT_t[(h, c)] = bbT

                    prev = None
                    for (h, ci) in hc:
                        emit_w1(h, ci)
                        if prev is not None:
                            emit_transp(*prev)
                        prev = (h, ci)
                    emit_transp(*prev)

                    for (h, c) in hc:
                        t0, t1 = TCH[c]
                        nc.scalar.activation(
                            eT[h][:, t0 * 80:t1 * 80],
                            bbT_t[(h, c)][:, :(t1 - t0) * 80], AF.Exp)
                    for (h, c) in hc:
                        t0, t1 = TCH[c]
                        ncc = t1 - t0
                        nc.vector.reduce_sum(
                            zrT[h][:, t0 * BLOC:t1 * BLOC].rearrange(
                                "p (ci b) -> p ci b", ci=ncc),
                            eT[h][:, t0 * 80:t1 * 80].rearrange(
                                "p (ci j b) -> p ci b j", ci=ncc, j=OC),
                            axis=AX.X)
                        nc.vector.reciprocal(
                            zrT[h][:, t0 * BLOC:t1 * BLOC],
                            zrT[h][:, t0 * BLOC:t1 * BLOC])
                    for (h, c) in hc:
                        t0, t1 = TCH[c]
                        ncc = t1 - t0
                        eng = nc.gpsimd if h == 1 else nc.vector
                        eng.tensor_tensor(
                            cT16[h][:, t0 * 80:t1 * 80].rearrange(
                                "p (ci j b) -> p ci j b", ci=ncc, j=OC),
                            eT[h][:, t0 * 80:t1 * 80].rearrange(
                                "p (ci j b) -> p ci j b", ci=ncc, j=OC),
                            zrT[h][:, t0 * BLOC:t1 * BLOC].rearrange(
                                "p (ci b) -> p ci b", ci=ncc)[:, :, None, :]
                            .broadcast_to([128, ncc, OC, BLOC]),
                            op=ALU.mult)

                    # ---- W4: PE weighted-sum with uT stationary; masked
                    # reduce extracts the b'==b diagonal ----
                    # each (j, piece) group opens and closes within its piece
                    # (CoreSim allows only one open group per psum bank);
                    # piece partials land at columns (j, c, b') of s_ps.
                    s_ps_t = {}
                    for h in range(H):
                        s_ps_t[h] = psS.tile([128, 3 * OC * BLOC], dt.float32,
                                             tag="sps", name="s_ps")
                    for (h, c) in hc:
                        t0, t1 = TCH[c]
                        for j in range(OC):
                            col = (j * 3 + c) * BLOC
                            for ci in range(t0, t1):
                                nc.tensor.matmul(
                                    s_ps_t[h][:, col:col + BLOC],
                                    uTview(h, j, ci),
                                    cT16[h][:, ci * 80 + j * BLOC:
                                            ci * 80 + (j + 1) * BLOC],
                                    start=(ci == t0), stop=(ci == t1 - 1))
                    for h in range(H):
                        nc.vector.tensor_tensor(mskd[h][:], s_ps_t[h][:],
                                                b82a3_sb[:], op=ALU.mult)
                        nc.vector.reduce_sum(
                            s_sb[:, h::2],
                            mskd[h][:].rearrange("p (j cb) -> p j cb", j=OC),
                            axis=AX.X)
                        if not last:
                            for j in range(OC):
                                build_sfat(j, h)
                        mini_squash(h)
                        if last:
                            # final: v = f * s, with f replicated from [80,1]
                            # to [(b,d), j] via jmask scale + b82aT matmul
                            nc.vector.tensor_scalar(
                                out=fj16[h][:], in0=jmask_sb[:],
                                scalar1=f80[h][:, 0:1], scalar2=None,
                                op0=ALU.mult)
                            f_ps = psB.tile([128, OC], dt.float32,
                                            tag="bank", name="f_ps")
                            nc.tensor.matmul(f_ps[:], b82aT16_sb[:],
                                             fj16[h][:], start=True, stop=True)
                            nc.vector.tensor_tensor(
                                v_sb[:, h::2], s_sb[:, h::2], f_ps[:],
                                op=ALU.mult)
                nc.sync.dma_start(out_d, v_sb[:])

    nc.compile()
    return nc


def _get_program(general_b):
    key = bool(general_b)
    if key not in _PROG_CACHE:
        _PROG_CACHE[key] = _build_program(key)
    return _PROG_CACHE[key]


def _prep_inputs(u_predict, b):
    """Host-side shard + layout transform. Returns (in_maps, general_b)."""
    general_b = bool(np.any(b != 0.0))
    consts = _build_consts()
    u16 = u_predict.astype(np.float16)
    u6 = u16.reshape(NCORES, H, BLOC, IC, OC, D)
    ut = np.ascontiguousarray(u6.transpose(0, 2, 5, 4, 1, 3))
    ut = ut.reshape(NCORES, 128, OC, H, NI)
    # uT[c, i_lo, h, j, ci*128 + p] = ut[c, p, j, h, ci*128 + i_lo]
    u5 = ut.reshape(NCORES, 128, OC, H, NC9, 128)
    uTt = np.ascontiguousarray(u5.transpose(0, 5, 3, 2, 4, 1))
    uTt = uTt.reshape(NCORES, 128, H, OC, NC9 * 128)

    extra = {}
    if general_b:
        bm = b.astype(np.float64)
        e = np.exp(bm - bm.max(axis=1, keepdims=True))
        c0 = (e / e.sum(axis=1, keepdims=True)).astype(np.float16)  # [IC, OC]
        c0rep = np.ascontiguousarray(
            np.broadcast_to(c0.T[None, :, :], (128, OC, NI))).astype(
                np.float16)
        bt = b.astype(np.float32).T  # [OC, NI]
        bb0 = np.ascontiguousarray(
            np.repeat(bt[:, None, :], BLOC, axis=1)).reshape(OC * BLOC, NI)
        extra = {"c0rep": c0rep, "bb0": bb0}

    in_maps = []
    for c in range(NCORES):
        m = {"u16": ut[c], "uT": uTt[c]}
        m.update(consts)
        m.update(extra)
        in_maps.append(m)
    return in_maps, general_b


def _gather_output(results):
    out = np.empty((B, OC, D), np.float32)
    for c in range(NCORES):
        v = results[c]["vout"]                  # [p=(bl,d), col=(j*2+h)]
        v4 = v.reshape(BLOC, D, OC, H)          # bl, d, j, h
        out[c * BL:(c + 1) * BL] = v4.transpose(3, 0, 2, 1).reshape(
            BL, OC, D)
    return out


def kernel(u_predict, b=None, **kw):
    u_predict = np.asarray(u_predict, dtype=np.float32)
    if b is None:
        b = np.zeros((IC, OC), np.float32)
    b = np.asarray(b, dtype=np.float32)
    in_maps, general_b = _prep_inputs(u_predict, b)
    nc = _get_program(general_b)

    if os.environ.get("BASS_KERNEL_SIM"):
        from concourse.bass_interp import CoreSim
        sim = CoreSim(nc, trace=False)
        for name, arr in in_maps[0].items():
            sim.tensor(name)[:] = arr
        sim.simulate(check_with_hw=False)
        v0 = np.array(sim.tensor("vout"))
        out = np.empty((B, OC, D), np.float32)
        v4 = v0.reshape(BLOC, D, OC, H)
        out[:BL] = v4.transpose(3, 0, 2, 1).reshape(BL, OC, D)
        return out  # NOTE: only core 0 valid in sim mode

    from concourse import bass_utils
    trace = bool(os.environ.get("BASS_KERNEL_TRACE"))
    res = bass_utils.run_bass_kernel_spmd(
        nc, in_maps, core_ids=list(range(NCORES)), trace=trace)
    kernel.last_results = res
    return _gather_output(res.results)
